# revision 16
# baseline (speedup 1.0000x reference)
"""MoE (8 routed experts, top-2, + shared expert) on 8 TRN2 NeuronCores.

Strategy: generic-slot expert parallelism. Host computes the gate (fp32
numpy, exactly mirroring the reference). Every core runs the same three
SwiGLU "slots" (capacities fixed at compile time); each slot instance
(core, slot) is bound at runtime to ONE weight set — a routed expert or
the shared expert — plus a token block and per-token combine weights
(cw = 1 for shared tokens, 0 for padding). A small planner packs the 8
experts and the 4096 shared tokens into the 24 slot instances so that
per-core capacity (and thus PE time) is minimized; an expert may be
split across several instances. Host scatters slot outputs back and
combines in bf16 expert order.

All tensors fed to the device are pre-arranged on host into
partition-major layouts so every DMA is contiguous per partition:
  activations/weights for matmul lhsT/rhs always have the contraction
  dim chunked as [pi=128, po, free].
"""

from functools import lru_cache

import numpy as np
import ml_dtypes

import concourse.mybir as mybir
from concourse import bacc
from concourse.tile import TileContext
from concourse import bass_utils

BF16 = mybir.dt.bfloat16
F32 = mybir.dt.float32

D = 2048          # model dim
I = 1408          # expert inter dim
E = 8             # routed experts
TOPK = 2
N_CORES = 8
DPO = D // 128    # 16 chunks of the model dim
IPO = I // 128    # 11 chunks of the inter dim

_BUILD_CACHE = {}


def _ceil8(x):
    return int(-(-x // 8) * 8)


def _c_blocks(C):
    """Split C columns into equal-ish blocks <= 512, multiples of 8."""
    nb = -(-C // 512)
    per = -(-C // (nb * 8)) * 8
    blocks = []
    off = 0
    while off < C:
        w = min(per, C - off)
        blocks.append((off, w))
        off += w
    return blocks


def _build(caps):
    """Build the per-core Bass kernel with one SwiGLU job per slot
    capacity in `caps` (processed in the given order). Same NEFF runs
    SPMD on all 8 cores."""
    nc = bacc.Bacc("TRN2", debug=False, enable_asserts=False,
                   num_devices=N_CORES, enable_partition_id=False)

    def din(name, shape, dt=BF16):
        return nc.dram_tensor(name, shape, dt, kind="ExternalInput").ap()

    def dout(name, shape, dt=BF16):
        return nc.dram_tensor(name, shape, dt, kind="ExternalOutput").ap()

    jdefs = [(f"s{j}", CJ) for j, CJ in enumerate(caps)]
    dram = {}
    for jname, CJ in jdefs:
        dram[jname] = {
            "x": din(f"x_{jname}", [128, DPO, CJ]),
            "w1": din(f"w1_{jname}", [IPO, 128, D]),
            "w3": din(f"w3_{jname}", [IPO, 128, D]),
            "w2": din(f"w2_{jname}", [DPO, 128, I]),
            "cw": din(f"cw_{jname}", [128, CJ], F32),
            "out": dout(f"y_{jname}", [128, DPO, CJ]),
        }

    Silu = mybir.ActivationFunctionType.Silu

    with TileContext(nc) as tc:
        with tc.tile_pool(name="main", bufs=1) as pool, \
             tc.tile_pool(name="psum", bufs=1, space="PSUM") as pp:
            # HAM prewarm: the PE clock sits at 1.2 GHz until ~3.4us of
            # sustained activity. Burn dummy matmuls on a zeroed tile while
            # the startup DMAs are in flight so the real stream runs warm.
            warm = pool.tile([128, 128], BF16, tag="warm", bufs=1, name="warm")
            nc.gpsimd.memset(warm[:], 0.0)
            wp = pp.tile([128, 128], F32, tag="ps", bufs=8, name="warm_ps")
            for _ in range(22):
                nc.tensor.matmul(wp[:], warm[:], warm[:], start=True, stop=True)

            for jidx, (jname, CJ) in enumerate(jdefs):
                last_job = jidx == len(jdefs) - 1
                dd = dram[jname]
                cbs = _c_blocks(CJ)
                x_sb = pool.tile([128, DPO, CJ], BF16, tag=f"x_{jname}",
                                 bufs=1, name=f"x_{jname}")
                # prefetch the first PRE i-chunks' w1/w3 alongside the x
                # stream so neither the per-d x wait nor the first i-chunk
                # boundaries stall the PE
                PRE = 5
                w13_pre = []
                for i in range(PRE):
                    w1_sb = pool.tile([128, DPO, 128], BF16, tag="w13",
                                      bufs=10, name=f"w1_{jname}_{i}")
                    w3_sb = pool.tile([128, DPO, 128], BF16, tag="w13",
                                      bufs=10, name=f"w3_{jname}_{i}")
                    w13_pre.append((w1_sb, w3_sb))
                w1_0, w3_0 = w13_pre[0]
                wdr = [dd[k][0].rearrange("p (a b) -> p a b", a=DPO)
                      for k in ("w1", "w3")]
                # startup issue split: gpsimd (idle at startup) streams the
                # x slices while sync streams the first weight chunks
                nc.sync.dma_start(w1_0[:, 0:4, :], wdr[0][:, 0:4, :])
                nc.gpsimd.dma_start(x_sb[:, 0, :], dd["x"][:, 0, :])
                nc.sync.dma_start(w3_0[:, 0:4, :], wdr[1][:, 0:4, :])
                for dsl in range(1, DPO):
                    nc.gpsimd.dma_start(x_sb[:, dsl, :], dd["x"][:, dsl, :])
                nc.sync.dma_start(w1_0[:, 4:, :], wdr[0][:, 4:, :])
                nc.sync.dma_start(w3_0[:, 4:, :], wdr[1][:, 4:, :])
                for nxt in range(1, PRE):
                    for wi, wk in enumerate(("w1", "w3")):
                        nc.sync.dma_start(
                            w13_pre[nxt][wi][:],
                            dd[wk][nxt].rearrange("p (a b) -> p a b", a=DPO))
                cw_sb = pool.tile([128, CJ], F32, tag=f"cw_{jname}",
                                  bufs=1, name=f"cw_{jname}")
                nc.gpsimd.dma_start(cw_sb[:], dd["cw"][:])
                H = pool.tile([128, IPO, CJ], BF16, tag=f"H_{jname}",
                              bufs=1, name=f"H_{jname}")

                # ---- phase A: H = silu(x@w1T) * (x@w3T) * cw ----
                def get_w13(i):
                    if i < PRE:
                        return w13_pre[i]
                    w1_sb = pool.tile([128, DPO, 128], BF16, tag="w13",
                                      bufs=10, name=f"w1_{jname}_{i}")
                    nc.sync.dma_start(
                        w1_sb[:],
                        dd["w1"][i].rearrange("p (a b) -> p a b", a=DPO))
                    w3_sb = pool.tile([128, DPO, 128], BF16, tag="w13",
                                      bufs=10, name=f"w3_{jname}_{i}")
                    nc.sync.dma_start(
                        w3_sb[:],
                        dd["w3"][i].rearrange("p (a b) -> p a b", a=DPO))
                    return w1_sb, w3_sb

                def act_block(i, p1s, p3s):
                    for bi, (off, w) in enumerate(cbs):
                        s_t = pool.tile([128, w], F32, tag="act1", bufs=6,
                                        name=f"s_{jname}_{i}_{bi}")
                        nc.scalar.activation(s_t[:], p1s[bi][:], Silu)
                        t_t = pool.tile([128, w], F32, tag="act2", bufs=6,
                                        name=f"t_{jname}_{i}_{bi}")
                        nc.vector.tensor_mul(t_t[:], p3s[bi][:],
                                             cw_sb[:, off:off + w])
                        nc.vector.tensor_mul(H[:, i, off:off + w],
                                             s_t[:], t_t[:])

                def psum_pair(i):
                    p1s = []
                    p3s = []
                    for bi, (off, w) in enumerate(cbs):
                        p1s.append(pp.tile([128, w], F32, tag="ps", bufs=8,
                                           name=f"p1_{jname}_{i}_{bi}"))
                        p3s.append(pp.tile([128, w], F32, tag="ps", bufs=8,
                                           name=f"p3_{jname}_{i}_{bi}"))
                    return p1s, p3s

                if jidx == 0 and len(cbs) == 1:
                    # first job: interleave i-chunk pairs so each arriving
                    # x slice feeds 2x the matmul work — halves the
                    # PE-idle time while x streams in over ~11us
                    ip = 0
                    while ip < IPO:
                        ii = [ip] if ip + 1 >= IPO else [ip, ip + 1]
                        ws = [get_w13(i) for i in ii]
                        ps = [psum_pair(i) for i in ii]
                        for d in range(DPO):
                            for (w1_sb, w3_sb), (p1s, p3s) in zip(ws, ps):
                                nc.tensor.matmul(
                                    p1s[0][:], w1_sb[:, d, :],
                                    x_sb[:, d, :],
                                    start=(d == 0), stop=(d == DPO - 1))
                                nc.tensor.matmul(
                                    p3s[0][:], w3_sb[:, d, :],
                                    x_sb[:, d, :],
                                    start=(d == 0), stop=(d == DPO - 1))
                        for i, (p1s, p3s) in zip(ii, ps):
                            act_block(i, p1s, p3s)
                        ip += len(ii)
                else:
                    for i in range(IPO):
                        w1_sb, w3_sb = get_w13(i)
                        p1s, p3s = psum_pair(i)
                        for d in range(DPO):
                            for bi, (off, w) in enumerate(cbs):
                                nc.tensor.matmul(
                                    p1s[bi][:], w1_sb[:, d, :],
                                    x_sb[:, d, off:off + w],
                                    start=(d == 0), stop=(d == DPO - 1))
                            for bi, (off, w) in enumerate(cbs):
                                nc.tensor.matmul(
                                    p3s[bi][:], w3_sb[:, d, :],
                                    x_sb[:, d, off:off + w],
                                    start=(d == 0), stop=(d == DPO - 1))
                        act_block(i, p1s, p3s)

                # ---- phase B: out = H @ w2T ----
                for do in range(DPO):
                    w2_sb = pool.tile([128, IPO, 128], BF16, tag="w2",
                                      bufs=6, name=f"w2_{jname}_{do}")
                    nc.sync.dma_start(
                        w2_sb[:],
                        dd["w2"][do].rearrange("p (a b) -> p a b", a=IPO))
                    pys = []
                    for bi, (off, w) in enumerate(cbs):
                        pys.append(pp.tile([128, w], F32, tag="ps", bufs=8,
                                           name=f"py_{jname}_{do}_{bi}"))
                    y_t = pool.tile([128, CJ], BF16, tag="yo", bufs=4,
                                    name=f"y_{jname}_{do}")
                    if last_job and do == DPO - 1 and len(cbs) > 1:
                        # tail: accumulate block-major so the cast of each
                        # finished block overlaps the next block's matmuls
                        for bi, (off, w) in enumerate(cbs):
                            for i in range(IPO):
                                nc.tensor.matmul(
                                    pys[bi][:], w2_sb[:, i, :],
                                    H[:, i, off:off + w],
                                    start=(i == 0), stop=(i == IPO - 1))
                            nc.vector.tensor_copy(y_t[:, off:off + w],
                                                  pys[bi][:])
                    else:
                        for i in range(IPO):
                            for bi, (off, w) in enumerate(cbs):
                                nc.tensor.matmul(
                                    pys[bi][:], w2_sb[:, i, :],
                                    H[:, i, off:off + w],
                                    start=(i == 0), stop=(i == IPO - 1))
                        for bi, (off, w) in enumerate(cbs):
                            nc.vector.tensor_copy(y_t[:, off:off + w],
                                                  pys[bi][:])
                    if last_job and do == DPO - 1:
                        # final transfer: issue from the (idle) scalar
                        # engine in case sync is backed up at kernel end
                        nc.scalar.dma_start(dd["out"][:, do, :], y_t[:])
                    else:
                        nc.sync.dma_start(dd["out"][:, do, :], y_t[:])

    nc.finalize()
    return nc


def _get_kernel(caps):
    key = tuple(caps)
    if key not in _BUILD_CACHE:
        _BUILD_CACHE[key] = _build(key)
    return _BUILD_CACHE[key]


def _pm(a, po):
    """[N, po*128] -> partition-major [128, po, N] contiguous."""
    n = a.shape[0]
    return np.ascontiguousarray(
        a.T.reshape(po, 128, n).transpose(1, 0, 2))


def _solve_caps(caps, counts, shared_total, maxslots=4):
    """DP: can the experts be packed into 8 instances of each cap (each
    instance single-expert) leaving >= shared_total capacity? Returns
    (expert order, per-expert slot usage) or None."""
    caps = tuple(caps)
    exps = sorted(range(len(counts)), key=lambda e: -counts[e])

    @lru_cache(maxsize=None)
    def rec(ei, a0, a1, a2):
        if ei == len(exps):
            rest = a0 * caps[0] + a1 * caps[1] + a2 * caps[2]
            return (0, ()) if rest >= shared_total else None
        c = counts[exps[ei]]
        best = None
        for n0 in range(0, min(a0, maxslots) + 1):
            for n1 in range(0, min(a1, maxslots) + 1):
                for n2 in range(0, min(a2, maxslots) + 1):
                    if not 1 <= n0 + n1 + n2 <= maxslots:
                        continue
                    s = n0 * caps[0] + n1 * caps[1] + n2 * caps[2]
                    if s < c:
                        continue
                    r = rec(ei + 1, a0 - n0, a1 - n1, a2 - n2)
                    if r is None:
                        continue
                    w = s - c + r[0]
                    if best is None or w < best[0]:
                        best = (w, ((n0, n1, n2),) + r[1])
        return best

    r = rec(0, 8, 8, 8)
    return (exps, r[1]) if r is not None else None


def _plan_slots(counts, shared_total):
    """Pick 3 slot capacities (multiples of 8, each >= 384 so weight
    streams stay comfortably under DMA bandwidth) minimizing per-core
    capacity, and assign experts + shared tokens to slot instances.

    Returns (caps, assign) with caps ascending (processing order) and
    assign[core] = [(slot_idx, item, lo, hi), ...]; item 0..E-1 = routed
    expert, item E = shared.
    """
    counts = [int(c) for c in counts]
    found = None
    for S in range(_ceil8((sum(counts) + shared_total) // N_CORES),
                   2200, 8):
        cands = []
        for c1 in range(392, min(1108, S - 2 * 384 + 1), 8):
            for c2 in range(384, c1 + 1, 8):
                c3 = S - c1 - c2
                if c3 < 384 or c3 > c2:
                    continue
                r = _solve_caps((c1, c2, c3), counts, shared_total)
                if r is not None:
                    waste = sum(
                        sum(n * c for n, c in zip(u, (c1, c2, c3)))
                        - counts[e] for e, u in zip(r[0], r[1]))
                    cands.append((waste, (c1, c2, c3), r))
        if cands:
            found = min(cands)
            break
    waste, caps, (exps, usage) = found
    # instantiate: 8 instances per cap; experts grab instances greedily
    free = {j: [(core, j) for core in range(N_CORES)] for j in range(3)}
    assign = [[] for _ in range(N_CORES)]
    for e, use in zip(exps, usage):
        pos = 0
        cnt = counts[e]
        insts = []
        for j in range(3):
            for _ in range(use[j]):
                insts.append(free[j].pop(0))
        # fill largest-cap instances first so every piece is contiguous
        insts.sort(key=lambda cj: -caps[cj[1]])
        for core, j in insts:
            take = min(caps[j], cnt - pos)
            assign[core].append((j, e, pos, pos + take))
            pos += take
        assert pos == cnt
    # shared fills every remaining instance
    pos = 0
    rem = [inst for j in range(3) for inst in free[j]]
    rem.sort()
    for core, j in rem:
        take = min(caps[j], shared_total - pos)
        assign[core].append((j, E, pos, pos + take))
        pos += take
    assert pos == shared_total
    # processing order = ascending capacity: remap slot indices
    order = sorted(range(3), key=lambda j: caps[j])
    remap = {j: order.index(j) for j in range(3)}
    caps_sorted = tuple(caps[j] for j in order)
    assign = [[(remap[j], e, lo, hi) for j, e, lo, hi in alist]
              for alist in assign]
    return caps_sorted, assign


def kernel(x, gate_w, gate_b, w1, w2, w3, sw1, sw2, sw3):
    bf16 = ml_dtypes.bfloat16
    x = np.asarray(x)
    gate_w = np.asarray(gate_w, dtype=np.float32)
    gate_b = np.asarray(gate_b, dtype=np.float32)
    w1 = np.asarray(w1)
    w2 = np.asarray(w2)
    w3 = np.asarray(w3)
    sw1 = np.asarray(sw1)
    sw2 = np.asarray(sw2)
    sw3 = np.asarray(sw3)

    B, S, Dx = x.shape
    assert Dx == D
    T = B * S
    xt = x.reshape(T, D)

    # ---- gate (fp32, mirrors reference: sqrt(softplus), top-2 on biased) ----
    xf = xt.astype(np.float32)
    logits = xf @ gate_w.T
    scores = np.sqrt(np.log1p(np.exp(-np.abs(logits)))
                     + np.maximum(logits, 0.0))
    biased = scores + gate_b
    idx = np.argsort(-biased, axis=1, kind="stable")[:, :TOPK]
    cw = np.zeros((T, E), dtype=np.float32)
    np.put_along_axis(cw, idx, np.take_along_axis(scores, idx, axis=1), axis=1)

    sel = np.zeros((T, E), dtype=bool)
    np.put_along_axis(sel, idx, True, axis=1)
    tok_lists = [np.nonzero(sel[:, e])[0] for e in range(E)]
    tok_lists.append(np.arange(T))          # item E = shared expert
    counts = np.array([len(t) for t in tok_lists[:E]])

    caps, assign = _plan_slots(counts, T)
    nc = _get_kernel(caps)

    # ---- per-core input prep ----
    # weight transforms: lhsT layouts, block-major so DMAs are contiguous
    def wA_layout(wm):  # [I, D] -> [IPO, 128, D]; [ib,pi,po*128+ic]
        return np.ascontiguousarray(
            wm.T.reshape(DPO, 128, IPO, 128).transpose(2, 1, 0, 3)
        ).reshape(IPO, 128, D)

    def wB_layout(wm):  # [D, I] -> [DPO, 128, I]; [db,pi,po*128+dc]
        return np.ascontiguousarray(
            wm.T.reshape(IPO, 128, DPO, 128).transpose(2, 1, 0, 3)
        ).reshape(DPO, 128, I)

    w1t = [wA_layout(w1[e]) for e in range(E)] + [wA_layout(sw1)]
    w3t = [wA_layout(w3[e]) for e in range(E)] + [wA_layout(sw3)]
    w2t = [wB_layout(w2[e]) for e in range(E)] + [wB_layout(sw2)]

    in_maps = []
    pieces = [[] for _ in range(E + 1)]
    for core in range(N_CORES):
        im = {}
        filled = set()
        for j, e, lo, hi in assign[core]:
            Cs = caps[j]
            cnt = hi - lo
            toks = tok_lists[e][lo:hi]
            xg = np.zeros((Cs, D), dtype=bf16)
            xg[:cnt] = xt[toks]
            cwe = np.zeros((Cs,), dtype=np.float32)
            cwe[:cnt] = cw[toks, e] if e < E else 1.0
            im[f"x_s{j}"] = _pm(xg, DPO)
            im[f"cw_s{j}"] = np.ascontiguousarray(
                np.broadcast_to(cwe[None, :], (128, Cs)))
            im[f"w1_s{j}"] = w1t[e]
            im[f"w3_s{j}"] = w3t[e]
            im[f"w2_s{j}"] = w2t[e]
            pieces[e].append((core, j, lo, cnt))
            filled.add(j)
        for j in range(3):
            if j not in filled:
                Cs = caps[j]
                im[f"x_s{j}"] = np.zeros((128, DPO, Cs), dtype=bf16)
                im[f"cw_s{j}"] = np.zeros((128, Cs), dtype=np.float32)
                im[f"w1_s{j}"] = w1t[E]
                im[f"w3_s{j}"] = w3t[E]
                im[f"w2_s{j}"] = w2t[E]
        in_maps.append(im)

    res = bass_utils.run_bass_kernel_spmd(
        nc, in_maps, core_ids=list(range(N_CORES)))
    global LAST_RESULT
    LAST_RESULT = res

    # ---- unshard + combine (bf16, reference addition order) ----
    y = np.zeros((T, D), dtype=bf16)
    z = np.zeros((T, D), dtype=bf16)
    for e in range(E + 1):
        toks = tok_lists[e]
        parts = []
        for core, j, lo, cnt in sorted(pieces[e], key=lambda p: p[2]):
            ye = res.results[core][f"y_s{j}"]             # [128, DPO, Cs]
            ye_tok = ye.transpose(2, 1, 0).reshape(caps[j], D)
            parts.append(ye_tok[:cnt])
        ye_all = np.concatenate(parts, axis=0) if len(parts) > 1 else parts[0]
        if e < E:
            y[toks] = y[toks] + ye_all
        else:
            z[toks] = ye_all
    out = (y + z).reshape(B, S, D)
    return out.astype(x.dtype)


# revision 18
# speedup vs baseline: 1.0230x; 1.0230x over previous
"""MoE (8 routed experts, top-2, + shared expert) on 8 TRN2 NeuronCores.

Strategy: generic-slot expert parallelism. Host computes the gate (fp32
numpy, exactly mirroring the reference). Every core runs the same three
SwiGLU "slots" (capacities fixed at compile time); each slot instance
(core, slot) is bound at runtime to ONE weight set — a routed expert or
the shared expert — plus a token block and per-token combine weights
(cw = 1 for shared tokens, 0 for padding). A small planner packs the 8
experts and the 4096 shared tokens into the 24 slot instances so that
per-core capacity (and thus PE time) is minimized; an expert may be
split across several instances. Host scatters slot outputs back and
combines in bf16 expert order.

All tensors fed to the device are pre-arranged on host into
partition-major layouts so every DMA is contiguous per partition:
  activations/weights for matmul lhsT/rhs always have the contraction
  dim chunked as [pi=128, po, free].
"""

from functools import lru_cache

import numpy as np
import ml_dtypes

import concourse.mybir as mybir
from concourse import bacc
from concourse.tile import TileContext
from concourse import bass_utils

BF16 = mybir.dt.bfloat16
F32 = mybir.dt.float32

D = 2048          # model dim
I = 1408          # expert inter dim
E = 8             # routed experts
TOPK = 2
N_CORES = 8
DPO = D // 128    # 16 chunks of the model dim
IPO = I // 128    # 11 chunks of the inter dim

_BUILD_CACHE = {}


def _ceil8(x):
    return int(-(-x // 8) * 8)


def _c_blocks(C):
    """Split C columns into equal-ish blocks <= 512, multiples of 8."""
    nb = -(-C // 512)
    per = -(-C // (nb * 8)) * 8
    blocks = []
    off = 0
    while off < C:
        w = min(per, C - off)
        blocks.append((off, w))
        off += w
    return blocks


def _build(caps):
    """Build the per-core Bass kernel with one SwiGLU job per slot
    capacity in `caps` (processed in the given order). Same NEFF runs
    SPMD on all 8 cores."""
    nc = bacc.Bacc("TRN2", debug=False, enable_asserts=False,
                   num_devices=N_CORES, enable_partition_id=False)

    def din(name, shape, dt=BF16):
        return nc.dram_tensor(name, shape, dt, kind="ExternalInput").ap()

    def dout(name, shape, dt=BF16):
        return nc.dram_tensor(name, shape, dt, kind="ExternalOutput").ap()

    jdefs = [(f"s{j}", CJ) for j, CJ in enumerate(caps)]
    dram = {}
    for jname, CJ in jdefs:
        dram[jname] = {
            "x": din(f"x_{jname}", [128, DPO, CJ]),
            "w1": din(f"w1_{jname}", [IPO, 128, D]),
            "w3": din(f"w3_{jname}", [IPO, 128, D]),
            "w2": din(f"w2_{jname}", [DPO, 128, I]),
            "cw": din(f"cw_{jname}", [128, CJ], F32),
            "out": dout(f"y_{jname}", [128, DPO, CJ]),
        }

    Silu = mybir.ActivationFunctionType.Silu

    with TileContext(nc) as tc:
        with tc.tile_pool(name="main", bufs=1) as pool, \
             tc.tile_pool(name="psum", bufs=1, space="PSUM") as pp:
            # HAM prewarm: the PE clock sits at 1.2 GHz until ~3.4us of
            # sustained activity. Burn dummy matmuls on a zeroed tile while
            # the startup DMAs are in flight so the real stream runs warm.
            warm = pool.tile([128, 128], BF16, tag="warm", bufs=1, name="warm")
            nc.gpsimd.memset(warm[:], 0.0)
            wp = pp.tile([128, 128], F32, tag="ps", bufs=8, name="warm_ps")
            for _ in range(22):
                nc.tensor.matmul(wp[:], warm[:], warm[:], start=True, stop=True)

            for jidx, (jname, CJ) in enumerate(jdefs):
                last_job = jidx == len(jdefs) - 1
                dd = dram[jname]
                cbs = _c_blocks(CJ)
                x_sb = pool.tile([128, DPO, CJ], BF16, tag=f"x_{jname}",
                                 bufs=1, name=f"x_{jname}")
                # prefetch the first PRE i-chunks' w1/w3 alongside the x
                # stream so neither the per-d x wait nor the first i-chunk
                # boundaries stall the PE
                PRE = 4
                w13_pre = []
                for i in range(PRE):
                    w1_sb = pool.tile([128, DPO, 128], BF16, tag="w13",
                                      bufs=8, name=f"w1_{jname}_{i}")
                    w3_sb = pool.tile([128, DPO, 128], BF16, tag="w13",
                                      bufs=8, name=f"w3_{jname}_{i}")
                    w13_pre.append((w1_sb, w3_sb))
                w1_0, w3_0 = w13_pre[0]
                wdr = [dd[k][0].rearrange("p (a b) -> p a b", a=DPO)
                      for k in ("w1", "w3")]
                # startup issue split: gpsimd (idle at startup) streams the
                # x slices while sync streams the first weight chunks
                nc.sync.dma_start(w1_0[:, 0:4, :], wdr[0][:, 0:4, :])
                nc.gpsimd.dma_start(x_sb[:, 0, :], dd["x"][:, 0, :])
                nc.sync.dma_start(w3_0[:, 0:4, :], wdr[1][:, 0:4, :])
                for dsl in range(1, DPO):
                    nc.gpsimd.dma_start(x_sb[:, dsl, :], dd["x"][:, dsl, :])
                nc.sync.dma_start(w1_0[:, 4:, :], wdr[0][:, 4:, :])
                nc.sync.dma_start(w3_0[:, 4:, :], wdr[1][:, 4:, :])
                for nxt in range(1, PRE):
                    for wi, wk in enumerate(("w1", "w3")):
                        nc.sync.dma_start(
                            w13_pre[nxt][wi][:],
                            dd[wk][nxt].rearrange("p (a b) -> p a b", a=DPO))
                cw_sb = pool.tile([128, CJ], F32, tag=f"cw_{jname}",
                                  bufs=1, name=f"cw_{jname}")
                nc.gpsimd.dma_start(cw_sb[:], dd["cw"][:])
                H = pool.tile([128, IPO, CJ], BF16, tag=f"H_{jname}",
                              bufs=1, name=f"H_{jname}")

                # ---- phase A: H = silu(x@w1T) * (x@w3T) * cw ----
                for i in range(IPO):
                    if i < PRE:
                        w1_sb, w3_sb = w13_pre[i]
                    else:
                        w1_sb = pool.tile([128, DPO, 128], BF16, tag="w13",
                                          bufs=8, name=f"w1_{jname}_{i}")
                        nc.sync.dma_start(
                            w1_sb[:],
                            dd["w1"][i].rearrange("p (a b) -> p a b", a=DPO))
                        w3_sb = pool.tile([128, DPO, 128], BF16, tag="w13",
                                          bufs=8, name=f"w3_{jname}_{i}")
                        nc.sync.dma_start(
                            w3_sb[:],
                            dd["w3"][i].rearrange("p (a b) -> p a b", a=DPO))
                    p1s = []
                    p3s = []
                    for bi, (off, w) in enumerate(cbs):
                        p1s.append(pp.tile([128, w], F32, tag="ps", bufs=8,
                                           name=f"p1_{jname}_{i}_{bi}"))
                        p3s.append(pp.tile([128, w], F32, tag="ps", bufs=8,
                                           name=f"p3_{jname}_{i}_{bi}"))
                    for d in range(DPO):
                        for bi, (off, w) in enumerate(cbs):
                            nc.tensor.matmul(
                                p1s[bi][:], w1_sb[:, d, :],
                                x_sb[:, d, off:off + w],
                                start=(d == 0), stop=(d == DPO - 1))
                        for bi, (off, w) in enumerate(cbs):
                            nc.tensor.matmul(
                                p3s[bi][:], w3_sb[:, d, :],
                                x_sb[:, d, off:off + w],
                                start=(d == 0), stop=(d == DPO - 1))
                    for bi, (off, w) in enumerate(cbs):
                        s_t = pool.tile([128, w], F32, tag="act1", bufs=6,
                                        name=f"s_{jname}_{i}_{bi}")
                        nc.scalar.activation(s_t[:], p1s[bi][:], Silu)
                        t_t = pool.tile([128, w], F32, tag="act2", bufs=6,
                                        name=f"t_{jname}_{i}_{bi}")
                        nc.vector.tensor_mul(t_t[:], p3s[bi][:],
                                             cw_sb[:, off:off + w])
                        nc.vector.tensor_mul(H[:, i, off:off + w],
                                             s_t[:], t_t[:])

                # ---- phase B: out = H @ w2T ----
                for do in range(DPO):
                    w2_sb = pool.tile([128, IPO, 128], BF16, tag="w2",
                                      bufs=6, name=f"w2_{jname}_{do}")
                    nc.sync.dma_start(
                        w2_sb[:],
                        dd["w2"][do].rearrange("p (a b) -> p a b", a=IPO))
                    pys = []
                    for bi, (off, w) in enumerate(cbs):
                        pys.append(pp.tile([128, w], F32, tag="ps", bufs=8,
                                           name=f"py_{jname}_{do}_{bi}"))
                    y_t = pool.tile([128, CJ], BF16, tag="yo", bufs=4,
                                    name=f"y_{jname}_{do}")
                    if last_job and do == DPO - 1 and len(cbs) > 1:
                        # tail: accumulate block-major so the cast of each
                        # finished block overlaps the next block's matmuls
                        for bi, (off, w) in enumerate(cbs):
                            for i in range(IPO):
                                nc.tensor.matmul(
                                    pys[bi][:], w2_sb[:, i, :],
                                    H[:, i, off:off + w],
                                    start=(i == 0), stop=(i == IPO - 1))
                            nc.vector.tensor_copy(y_t[:, off:off + w],
                                                  pys[bi][:])
                    else:
                        for i in range(IPO):
                            for bi, (off, w) in enumerate(cbs):
                                nc.tensor.matmul(
                                    pys[bi][:], w2_sb[:, i, :],
                                    H[:, i, off:off + w],
                                    start=(i == 0), stop=(i == IPO - 1))
                        for bi, (off, w) in enumerate(cbs):
                            nc.vector.tensor_copy(y_t[:, off:off + w],
                                                  pys[bi][:])
                    if last_job and do == DPO - 1:
                        # final transfer: issue from the (idle) scalar
                        # engine in case sync is backed up at kernel end
                        nc.scalar.dma_start(dd["out"][:, do, :], y_t[:])
                    else:
                        nc.sync.dma_start(dd["out"][:, do, :], y_t[:])

    nc.finalize()
    return nc


def _get_kernel(caps):
    key = tuple(caps)
    if key not in _BUILD_CACHE:
        _BUILD_CACHE[key] = _build(key)
    return _BUILD_CACHE[key]


def _pm(a, po):
    """[N, po*128] -> partition-major [128, po, N] contiguous."""
    n = a.shape[0]
    return np.ascontiguousarray(
        a.T.reshape(po, 128, n).transpose(1, 0, 2))


def _solve_caps(caps, counts, shared_total, maxslots=4):
    """DP: can the experts be packed into 8 instances of each cap (each
    instance single-expert) leaving >= shared_total capacity? Returns
    (expert order, per-expert slot usage) or None."""
    caps = tuple(caps)
    exps = sorted(range(len(counts)), key=lambda e: -counts[e])

    @lru_cache(maxsize=None)
    def rec(ei, a0, a1, a2):
        if ei == len(exps):
            rest = a0 * caps[0] + a1 * caps[1] + a2 * caps[2]
            return (0, ()) if rest >= shared_total else None
        c = counts[exps[ei]]
        best = None
        for n0 in range(0, min(a0, maxslots) + 1):
            for n1 in range(0, min(a1, maxslots) + 1):
                for n2 in range(0, min(a2, maxslots) + 1):
                    if not 1 <= n0 + n1 + n2 <= maxslots:
                        continue
                    s = n0 * caps[0] + n1 * caps[1] + n2 * caps[2]
                    if s < c:
                        continue
                    r = rec(ei + 1, a0 - n0, a1 - n1, a2 - n2)
                    if r is None:
                        continue
                    w = s - c + r[0]
                    if best is None or w < best[0]:
                        best = (w, ((n0, n1, n2),) + r[1])
        return best

    r = rec(0, 8, 8, 8)
    return (exps, r[1]) if r is not None else None


def _plan_slots(counts, shared_total):
    """Pick 3 slot capacities (multiples of 8, each >= 384 so weight
    streams stay comfortably under DMA bandwidth) minimizing per-core
    capacity, and assign experts + shared tokens to slot instances.

    Returns (caps, assign) with caps ascending (processing order) and
    assign[core] = [(slot_idx, item, lo, hi), ...]; item 0..E-1 = routed
    expert, item E = shared.
    """
    counts = [int(c) for c in counts]
    found = None
    for S in range(_ceil8((sum(counts) + shared_total) // N_CORES),
                   2200, 8):
        cands = []
        for c1 in range(392, min(1108, S - 2 * 384 + 1), 8):
            for c2 in range(384, c1 + 1, 8):
                c3 = S - c1 - c2
                if c3 < 384 or c3 > c2:
                    continue
                r = _solve_caps((c1, c2, c3), counts, shared_total)
                if r is not None:
                    waste = sum(
                        sum(n * c for n, c in zip(u, (c1, c2, c3)))
                        - counts[e] for e, u in zip(r[0], r[1]))
                    cands.append((waste, (c1, c2, c3), r))
        if cands:
            found = min(cands)
            break
    waste, caps, (exps, usage) = found
    # instantiate: 8 instances per cap; experts grab instances greedily
    free = {j: [(core, j) for core in range(N_CORES)] for j in range(3)}
    assign = [[] for _ in range(N_CORES)]
    for e, use in zip(exps, usage):
        pos = 0
        cnt = counts[e]
        insts = []
        for j in range(3):
            for _ in range(use[j]):
                insts.append(free[j].pop(0))
        # fill largest-cap instances first so every piece is contiguous
        insts.sort(key=lambda cj: -caps[cj[1]])
        for core, j in insts:
            take = min(caps[j], cnt - pos)
            assign[core].append((j, e, pos, pos + take))
            pos += take
        assert pos == cnt
    # shared fills every remaining instance
    pos = 0
    rem = [inst for j in range(3) for inst in free[j]]
    rem.sort()
    for core, j in rem:
        take = min(caps[j], shared_total - pos)
        assign[core].append((j, E, pos, pos + take))
        pos += take
    assert pos == shared_total
    # processing order = ascending capacity: remap slot indices
    order = sorted(range(3), key=lambda j: caps[j])
    remap = {j: order.index(j) for j in range(3)}
    caps_sorted = tuple(caps[j] for j in order)
    assign = [[(remap[j], e, lo, hi) for j, e, lo, hi in alist]
              for alist in assign]
    return caps_sorted, assign


def kernel(x, gate_w, gate_b, w1, w2, w3, sw1, sw2, sw3):
    bf16 = ml_dtypes.bfloat16
    x = np.asarray(x)
    gate_w = np.asarray(gate_w, dtype=np.float32)
    gate_b = np.asarray(gate_b, dtype=np.float32)
    w1 = np.asarray(w1)
    w2 = np.asarray(w2)
    w3 = np.asarray(w3)
    sw1 = np.asarray(sw1)
    sw2 = np.asarray(sw2)
    sw3 = np.asarray(sw3)

    B, S, Dx = x.shape
    assert Dx == D
    T = B * S
    xt = x.reshape(T, D)

    # ---- gate (fp32, mirrors reference: sqrt(softplus), top-2 on biased) ----
    xf = xt.astype(np.float32)
    logits = xf @ gate_w.T
    scores = np.sqrt(np.log1p(np.exp(-np.abs(logits)))
                     + np.maximum(logits, 0.0))
    biased = scores + gate_b
    idx = np.argsort(-biased, axis=1, kind="stable")[:, :TOPK]
    cw = np.zeros((T, E), dtype=np.float32)
    np.put_along_axis(cw, idx, np.take_along_axis(scores, idx, axis=1), axis=1)

    sel = np.zeros((T, E), dtype=bool)
    np.put_along_axis(sel, idx, True, axis=1)
    tok_lists = [np.nonzero(sel[:, e])[0] for e in range(E)]
    tok_lists.append(np.arange(T))          # item E = shared expert
    counts = np.array([len(t) for t in tok_lists[:E]])

    caps, assign = _plan_slots(counts, T)
    nc = _get_kernel(caps)

    # ---- per-core input prep ----
    # weight transforms: lhsT layouts, block-major so DMAs are contiguous
    def wA_layout(wm):  # [I, D] -> [IPO, 128, D]; [ib,pi,po*128+ic]
        return np.ascontiguousarray(
            wm.T.reshape(DPO, 128, IPO, 128).transpose(2, 1, 0, 3)
        ).reshape(IPO, 128, D)

    def wB_layout(wm):  # [D, I] -> [DPO, 128, I]; [db,pi,po*128+dc]
        return np.ascontiguousarray(
            wm.T.reshape(IPO, 128, DPO, 128).transpose(2, 1, 0, 3)
        ).reshape(DPO, 128, I)

    w1t = [wA_layout(w1[e]) for e in range(E)] + [wA_layout(sw1)]
    w3t = [wA_layout(w3[e]) for e in range(E)] + [wA_layout(sw3)]
    w2t = [wB_layout(w2[e]) for e in range(E)] + [wB_layout(sw2)]

    in_maps = []
    pieces = [[] for _ in range(E + 1)]
    for core in range(N_CORES):
        im = {}
        filled = set()
        for j, e, lo, hi in assign[core]:
            Cs = caps[j]
            cnt = hi - lo
            toks = tok_lists[e][lo:hi]
            xg = np.zeros((Cs, D), dtype=bf16)
            xg[:cnt] = xt[toks]
            cwe = np.zeros((Cs,), dtype=np.float32)
            cwe[:cnt] = cw[toks, e] if e < E else 1.0
            im[f"x_s{j}"] = _pm(xg, DPO)
            im[f"cw_s{j}"] = np.ascontiguousarray(
                np.broadcast_to(cwe[None, :], (128, Cs)))
            im[f"w1_s{j}"] = w1t[e]
            im[f"w3_s{j}"] = w3t[e]
            im[f"w2_s{j}"] = w2t[e]
            pieces[e].append((core, j, lo, cnt))
            filled.add(j)
        for j in range(3):
            if j not in filled:
                Cs = caps[j]
                im[f"x_s{j}"] = np.zeros((128, DPO, Cs), dtype=bf16)
                im[f"cw_s{j}"] = np.zeros((128, Cs), dtype=np.float32)
                im[f"w1_s{j}"] = w1t[E]
                im[f"w3_s{j}"] = w3t[E]
                im[f"w2_s{j}"] = w2t[E]
        in_maps.append(im)

    res = bass_utils.run_bass_kernel_spmd(
        nc, in_maps, core_ids=list(range(N_CORES)))
    global LAST_RESULT
    LAST_RESULT = res

    # ---- unshard + combine (bf16, reference addition order) ----
    y = np.zeros((T, D), dtype=bf16)
    z = np.zeros((T, D), dtype=bf16)
    for e in range(E + 1):
        toks = tok_lists[e]
        parts = []
        for core, j, lo, cnt in sorted(pieces[e], key=lambda p: p[2]):
            ye = res.results[core][f"y_s{j}"]             # [128, DPO, Cs]
            ye_tok = ye.transpose(2, 1, 0).reshape(caps[j], D)
            parts.append(ye_tok[:cnt])
        ye_all = np.concatenate(parts, axis=0) if len(parts) > 1 else parts[0]
        if e < E:
            y[toks] = y[toks] + ye_all
        else:
            z[toks] = ye_all
    out = (y + z).reshape(B, S, D)
    return out.astype(x.dtype)


# revision 19
# speedup vs baseline: 1.0263x; 1.0032x over previous
"""MoE (8 routed experts, top-2, + shared expert) on 8 TRN2 NeuronCores.

Strategy: generic-slot expert parallelism. Host computes the gate (fp32
numpy, exactly mirroring the reference). Every core runs the same three
SwiGLU "slots" (capacities fixed at compile time); each slot instance
(core, slot) is bound at runtime to ONE weight set — a routed expert or
the shared expert — plus a token block and per-token combine weights
(cw = 1 for shared tokens, 0 for padding). A small planner packs the 8
experts and the 4096 shared tokens into the 24 slot instances so that
per-core capacity (and thus PE time) is minimized; an expert may be
split across several instances. Host scatters slot outputs back and
combines in bf16 expert order.

All tensors fed to the device are pre-arranged on host into
partition-major layouts so every DMA is contiguous per partition:
  activations/weights for matmul lhsT/rhs always have the contraction
  dim chunked as [pi=128, po, free].
"""

from functools import lru_cache

import numpy as np
import ml_dtypes

import concourse.mybir as mybir
from concourse import bacc
from concourse.tile import TileContext
from concourse import bass_utils

BF16 = mybir.dt.bfloat16
F32 = mybir.dt.float32

D = 2048          # model dim
I = 1408          # expert inter dim
E = 8             # routed experts
TOPK = 2
N_CORES = 8
DPO = D // 128    # 16 chunks of the model dim
IPO = I // 128    # 11 chunks of the inter dim

_BUILD_CACHE = {}


def _ceil8(x):
    return int(-(-x // 8) * 8)


def _c_blocks(C):
    """Split C columns into equal-ish blocks <= 512, multiples of 8."""
    nb = -(-C // 512)
    per = -(-C // (nb * 8)) * 8
    blocks = []
    off = 0
    while off < C:
        w = min(per, C - off)
        blocks.append((off, w))
        off += w
    return blocks


def _build(caps):
    """Build the per-core Bass kernel with one SwiGLU job per slot
    capacity in `caps` (processed in the given order). Same NEFF runs
    SPMD on all 8 cores."""
    nc = bacc.Bacc("TRN2", debug=False, enable_asserts=False,
                   num_devices=N_CORES, enable_partition_id=False)

    def din(name, shape, dt=BF16):
        return nc.dram_tensor(name, shape, dt, kind="ExternalInput").ap()

    def dout(name, shape, dt=BF16):
        return nc.dram_tensor(name, shape, dt, kind="ExternalOutput").ap()

    jdefs = [(f"s{j}", CJ) for j, CJ in enumerate(caps)]
    dram = {}
    for jname, CJ in jdefs:
        dram[jname] = {
            "x": din(f"x_{jname}", [128, DPO, CJ]),
            "w1": din(f"w1_{jname}", [IPO, 128, D]),
            "w3": din(f"w3_{jname}", [IPO, 128, D]),
            "w2": din(f"w2_{jname}", [DPO, 128, I]),
            "cw": din(f"cw_{jname}", [128, CJ], F32),
            "out": dout(f"y_{jname}", [128, DPO, CJ]),
        }

    Silu = mybir.ActivationFunctionType.Silu

    with TileContext(nc) as tc:
        with tc.tile_pool(name="main", bufs=1) as pool, \
             tc.tile_pool(name="psum", bufs=1, space="PSUM") as pp:
            # HAM prewarm: the PE clock sits at 1.2 GHz until ~3.4us of
            # sustained activity. Burn dummy matmuls on a zeroed tile while
            # the startup DMAs are in flight so the real stream runs warm.
            warm = pool.tile([128, 128], BF16, tag="warm", bufs=1, name="warm")
            nc.gpsimd.memset(warm[:], 0.0)
            wp = pp.tile([128, 128], F32, tag="ps", bufs=8, name="warm_ps")
            for _ in range(22):
                nc.tensor.matmul(wp[:], warm[:], warm[:], start=True, stop=True)

            for jidx, (jname, CJ) in enumerate(jdefs):
                last_job = jidx == len(jdefs) - 1
                dd = dram[jname]
                cbs = _c_blocks(CJ)
                x_sb = pool.tile([128, DPO, CJ], BF16, tag=f"x_{jname}",
                                 bufs=1, name=f"x_{jname}")
                # prefetch the first PRE i-chunks' w1/w3 alongside the x
                # stream so neither the per-d x wait nor the first i-chunk
                # boundaries stall the PE
                PRE = 4
                w13_pre = []
                for i in range(PRE):
                    w1_sb = pool.tile([128, DPO, 128], BF16, tag="w13",
                                      bufs=8, name=f"w1_{jname}_{i}")
                    w3_sb = pool.tile([128, DPO, 128], BF16, tag="w13",
                                      bufs=8, name=f"w3_{jname}_{i}")
                    w13_pre.append((w1_sb, w3_sb))
                w1_0, w3_0 = w13_pre[0]
                wdr = [dd[k][0].rearrange("p (a b) -> p a b", a=DPO)
                      for k in ("w1", "w3")]
                # Startup issue split for the FIRST job only: gpsimd (idle)
                # streams its x slices in parallel with sync streaming the
                # first weight chunks. Later jobs issue everything from
                # sync — program order then paces their x streams behind
                # the running job's weight stream, instead of flooding the
                # DMA during startup and starving job 0's w13 chunks
                # (observed as PE stalls at ~60-70us with an 800 GB/s
                # catch-up burst on the weight queue).
                xq = nc.gpsimd if jidx == 0 else nc.sync
                nc.sync.dma_start(w1_0[:, 0:4, :], wdr[0][:, 0:4, :])
                xq.dma_start(x_sb[:, 0, :], dd["x"][:, 0, :])
                nc.sync.dma_start(w3_0[:, 0:4, :], wdr[1][:, 0:4, :])
                for dsl in range(1, 4):
                    xq.dma_start(x_sb[:, dsl, :], dd["x"][:, dsl, :])
                nc.sync.dma_start(w1_0[:, 4:, :], wdr[0][:, 4:, :])
                nc.sync.dma_start(w3_0[:, 4:, :], wdr[1][:, 4:, :])
                nxt = 1
                for dsl in range(4, DPO):
                    xq.dma_start(x_sb[:, dsl, :], dd["x"][:, dsl, :])
                    if dsl % 4 == 0 and nxt < PRE:
                        for wi, wk in enumerate(("w1", "w3")):
                            nc.sync.dma_start(
                                w13_pre[nxt][wi][:],
                                dd[wk][nxt].rearrange("p (a b) -> p a b",
                                                      a=DPO))
                        nxt += 1
                while nxt < PRE:
                    for wi, wk in enumerate(("w1", "w3")):
                        nc.sync.dma_start(
                            w13_pre[nxt][wi][:],
                            dd[wk][nxt].rearrange("p (a b) -> p a b", a=DPO))
                    nxt += 1
                cw_sb = pool.tile([128, CJ], F32, tag=f"cw_{jname}",
                                  bufs=1, name=f"cw_{jname}")
                xq.dma_start(cw_sb[:], dd["cw"][:])
                H = pool.tile([128, IPO, CJ], BF16, tag=f"H_{jname}",
                              bufs=1, name=f"H_{jname}")

                # ---- phase A: H = silu(x@w1T) * (x@w3T) * cw ----
                for i in range(IPO):
                    if i < PRE:
                        w1_sb, w3_sb = w13_pre[i]
                    else:
                        w1_sb = pool.tile([128, DPO, 128], BF16, tag="w13",
                                          bufs=8, name=f"w1_{jname}_{i}")
                        nc.sync.dma_start(
                            w1_sb[:],
                            dd["w1"][i].rearrange("p (a b) -> p a b", a=DPO))
                        w3_sb = pool.tile([128, DPO, 128], BF16, tag="w13",
                                          bufs=8, name=f"w3_{jname}_{i}")
                        nc.sync.dma_start(
                            w3_sb[:],
                            dd["w3"][i].rearrange("p (a b) -> p a b", a=DPO))
                    p1s = []
                    p3s = []
                    for bi, (off, w) in enumerate(cbs):
                        p1s.append(pp.tile([128, w], F32, tag="ps", bufs=8,
                                           name=f"p1_{jname}_{i}_{bi}"))
                        p3s.append(pp.tile([128, w], F32, tag="ps", bufs=8,
                                           name=f"p3_{jname}_{i}_{bi}"))
                    for d in range(DPO):
                        for bi, (off, w) in enumerate(cbs):
                            nc.tensor.matmul(
                                p1s[bi][:], w1_sb[:, d, :],
                                x_sb[:, d, off:off + w],
                                start=(d == 0), stop=(d == DPO - 1))
                        for bi, (off, w) in enumerate(cbs):
                            nc.tensor.matmul(
                                p3s[bi][:], w3_sb[:, d, :],
                                x_sb[:, d, off:off + w],
                                start=(d == 0), stop=(d == DPO - 1))
                    for bi, (off, w) in enumerate(cbs):
                        s_t = pool.tile([128, w], F32, tag="act1", bufs=6,
                                        name=f"s_{jname}_{i}_{bi}")
                        nc.scalar.activation(s_t[:], p1s[bi][:], Silu)
                        t_t = pool.tile([128, w], F32, tag="act2", bufs=6,
                                        name=f"t_{jname}_{i}_{bi}")
                        nc.vector.tensor_mul(t_t[:], p3s[bi][:],
                                             cw_sb[:, off:off + w])
                        nc.vector.tensor_mul(H[:, i, off:off + w],
                                             s_t[:], t_t[:])

                # ---- phase B: out = H @ w2T ----
                for do in range(DPO):
                    w2_sb = pool.tile([128, IPO, 128], BF16, tag="w2",
                                      bufs=6, name=f"w2_{jname}_{do}")
                    nc.sync.dma_start(
                        w2_sb[:],
                        dd["w2"][do].rearrange("p (a b) -> p a b", a=IPO))
                    pys = []
                    for bi, (off, w) in enumerate(cbs):
                        pys.append(pp.tile([128, w], F32, tag="ps", bufs=8,
                                           name=f"py_{jname}_{do}_{bi}"))
                    y_t = pool.tile([128, CJ], BF16, tag="yo", bufs=4,
                                    name=f"y_{jname}_{do}")
                    if last_job and do == DPO - 1 and len(cbs) > 1:
                        # tail: accumulate block-major so the cast of each
                        # finished block overlaps the next block's matmuls
                        for bi, (off, w) in enumerate(cbs):
                            for i in range(IPO):
                                nc.tensor.matmul(
                                    pys[bi][:], w2_sb[:, i, :],
                                    H[:, i, off:off + w],
                                    start=(i == 0), stop=(i == IPO - 1))
                            nc.vector.tensor_copy(y_t[:, off:off + w],
                                                  pys[bi][:])
                    else:
                        for i in range(IPO):
                            for bi, (off, w) in enumerate(cbs):
                                nc.tensor.matmul(
                                    pys[bi][:], w2_sb[:, i, :],
                                    H[:, i, off:off + w],
                                    start=(i == 0), stop=(i == IPO - 1))
                        for bi, (off, w) in enumerate(cbs):
                            nc.vector.tensor_copy(y_t[:, off:off + w],
                                                  pys[bi][:])
                    if last_job and do == DPO - 1:
                        # final transfer: issue from the (idle) scalar
                        # engine in case sync is backed up at kernel end
                        nc.scalar.dma_start(dd["out"][:, do, :], y_t[:])
                    else:
                        nc.sync.dma_start(dd["out"][:, do, :], y_t[:])

    nc.finalize()
    return nc


def _get_kernel(caps):
    key = tuple(caps)
    if key not in _BUILD_CACHE:
        _BUILD_CACHE[key] = _build(key)
    return _BUILD_CACHE[key]


def _pm(a, po):
    """[N, po*128] -> partition-major [128, po, N] contiguous."""
    n = a.shape[0]
    return np.ascontiguousarray(
        a.T.reshape(po, 128, n).transpose(1, 0, 2))


def _solve_caps(caps, counts, shared_total, maxslots=4):
    """DP: can the experts be packed into 8 instances of each cap (each
    instance single-expert) leaving >= shared_total capacity? Returns
    (expert order, per-expert slot usage) or None."""
    caps = tuple(caps)
    exps = sorted(range(len(counts)), key=lambda e: -counts[e])

    @lru_cache(maxsize=None)
    def rec(ei, a0, a1, a2):
        if ei == len(exps):
            rest = a0 * caps[0] + a1 * caps[1] + a2 * caps[2]
            return (0, ()) if rest >= shared_total else None
        c = counts[exps[ei]]
        best = None
        for n0 in range(0, min(a0, maxslots) + 1):
            for n1 in range(0, min(a1, maxslots) + 1):
                for n2 in range(0, min(a2, maxslots) + 1):
                    if not 1 <= n0 + n1 + n2 <= maxslots:
                        continue
                    s = n0 * caps[0] + n1 * caps[1] + n2 * caps[2]
                    if s < c:
                        continue
                    r = rec(ei + 1, a0 - n0, a1 - n1, a2 - n2)
                    if r is None:
                        continue
                    w = s - c + r[0]
                    if best is None or w < best[0]:
                        best = (w, ((n0, n1, n2),) + r[1])
        return best

    r = rec(0, 8, 8, 8)
    return (exps, r[1]) if r is not None else None


def _plan_slots(counts, shared_total):
    """Pick 3 slot capacities (multiples of 8, each >= 384 so weight
    streams stay comfortably under DMA bandwidth) minimizing per-core
    capacity, and assign experts + shared tokens to slot instances.

    Returns (caps, assign) with caps ascending (processing order) and
    assign[core] = [(slot_idx, item, lo, hi), ...]; item 0..E-1 = routed
    expert, item E = shared.
    """
    counts = [int(c) for c in counts]
    found = None
    for S in range(_ceil8((sum(counts) + shared_total) // N_CORES),
                   2200, 8):
        cands = []
        for c1 in range(392, min(1108, S - 2 * 384 + 1), 8):
            for c2 in range(384, c1 + 1, 8):
                c3 = S - c1 - c2
                if c3 < 384 or c3 > c2:
                    continue
                r = _solve_caps((c1, c2, c3), counts, shared_total)
                if r is not None:
                    waste = sum(
                        sum(n * c for n, c in zip(u, (c1, c2, c3)))
                        - counts[e] for e, u in zip(r[0], r[1]))
                    cands.append((waste, (c1, c2, c3), r))
        if cands:
            found = min(cands)
            break
    waste, caps, (exps, usage) = found
    # instantiate: 8 instances per cap; experts grab instances greedily
    free = {j: [(core, j) for core in range(N_CORES)] for j in range(3)}
    assign = [[] for _ in range(N_CORES)]
    for e, use in zip(exps, usage):
        pos = 0
        cnt = counts[e]
        insts = []
        for j in range(3):
            for _ in range(use[j]):
                insts.append(free[j].pop(0))
        # fill largest-cap instances first so every piece is contiguous
        insts.sort(key=lambda cj: -caps[cj[1]])
        for core, j in insts:
            take = min(caps[j], cnt - pos)
            assign[core].append((j, e, pos, pos + take))
            pos += take
        assert pos == cnt
    # shared fills every remaining instance
    pos = 0
    rem = [inst for j in range(3) for inst in free[j]]
    rem.sort()
    for core, j in rem:
        take = min(caps[j], shared_total - pos)
        assign[core].append((j, E, pos, pos + take))
        pos += take
    assert pos == shared_total
    # processing order = ascending capacity: remap slot indices
    order = sorted(range(3), key=lambda j: caps[j])
    remap = {j: order.index(j) for j in range(3)}
    caps_sorted = tuple(caps[j] for j in order)
    assign = [[(remap[j], e, lo, hi) for j, e, lo, hi in alist]
              for alist in assign]
    return caps_sorted, assign


def kernel(x, gate_w, gate_b, w1, w2, w3, sw1, sw2, sw3):
    bf16 = ml_dtypes.bfloat16
    x = np.asarray(x)
    gate_w = np.asarray(gate_w, dtype=np.float32)
    gate_b = np.asarray(gate_b, dtype=np.float32)
    w1 = np.asarray(w1)
    w2 = np.asarray(w2)
    w3 = np.asarray(w3)
    sw1 = np.asarray(sw1)
    sw2 = np.asarray(sw2)
    sw3 = np.asarray(sw3)

    B, S, Dx = x.shape
    assert Dx == D
    T = B * S
    xt = x.reshape(T, D)

    # ---- gate (fp32, mirrors reference: sqrt(softplus), top-2 on biased) ----
    xf = xt.astype(np.float32)
    logits = xf @ gate_w.T
    scores = np.sqrt(np.log1p(np.exp(-np.abs(logits)))
                     + np.maximum(logits, 0.0))
    biased = scores + gate_b
    idx = np.argsort(-biased, axis=1, kind="stable")[:, :TOPK]
    cw = np.zeros((T, E), dtype=np.float32)
    np.put_along_axis(cw, idx, np.take_along_axis(scores, idx, axis=1), axis=1)

    sel = np.zeros((T, E), dtype=bool)
    np.put_along_axis(sel, idx, True, axis=1)
    tok_lists = [np.nonzero(sel[:, e])[0] for e in range(E)]
    tok_lists.append(np.arange(T))          # item E = shared expert
    counts = np.array([len(t) for t in tok_lists[:E]])

    caps, assign = _plan_slots(counts, T)
    nc = _get_kernel(caps)

    # ---- per-core input prep ----
    # weight transforms: lhsT layouts, block-major so DMAs are contiguous
    def wA_layout(wm):  # [I, D] -> [IPO, 128, D]; [ib,pi,po*128+ic]
        return np.ascontiguousarray(
            wm.T.reshape(DPO, 128, IPO, 128).transpose(2, 1, 0, 3)
        ).reshape(IPO, 128, D)

    def wB_layout(wm):  # [D, I] -> [DPO, 128, I]; [db,pi,po*128+dc]
        return np.ascontiguousarray(
            wm.T.reshape(IPO, 128, DPO, 128).transpose(2, 1, 0, 3)
        ).reshape(DPO, 128, I)

    w1t = [wA_layout(w1[e]) for e in range(E)] + [wA_layout(sw1)]
    w3t = [wA_layout(w3[e]) for e in range(E)] + [wA_layout(sw3)]
    w2t = [wB_layout(w2[e]) for e in range(E)] + [wB_layout(sw2)]

    in_maps = []
    pieces = [[] for _ in range(E + 1)]
    for core in range(N_CORES):
        im = {}
        filled = set()
        for j, e, lo, hi in assign[core]:
            Cs = caps[j]
            cnt = hi - lo
            toks = tok_lists[e][lo:hi]
            xg = np.zeros((Cs, D), dtype=bf16)
            xg[:cnt] = xt[toks]
            cwe = np.zeros((Cs,), dtype=np.float32)
            cwe[:cnt] = cw[toks, e] if e < E else 1.0
            im[f"x_s{j}"] = _pm(xg, DPO)
            im[f"cw_s{j}"] = np.ascontiguousarray(
                np.broadcast_to(cwe[None, :], (128, Cs)))
            im[f"w1_s{j}"] = w1t[e]
            im[f"w3_s{j}"] = w3t[e]
            im[f"w2_s{j}"] = w2t[e]
            pieces[e].append((core, j, lo, cnt))
            filled.add(j)
        for j in range(3):
            if j not in filled:
                Cs = caps[j]
                im[f"x_s{j}"] = np.zeros((128, DPO, Cs), dtype=bf16)
                im[f"cw_s{j}"] = np.zeros((128, Cs), dtype=np.float32)
                im[f"w1_s{j}"] = w1t[E]
                im[f"w3_s{j}"] = w3t[E]
                im[f"w2_s{j}"] = w2t[E]
        in_maps.append(im)

    res = bass_utils.run_bass_kernel_spmd(
        nc, in_maps, core_ids=list(range(N_CORES)))
    global LAST_RESULT
    LAST_RESULT = res

    # ---- unshard + combine (bf16, reference addition order) ----
    y = np.zeros((T, D), dtype=bf16)
    z = np.zeros((T, D), dtype=bf16)
    for e in range(E + 1):
        toks = tok_lists[e]
        parts = []
        for core, j, lo, cnt in sorted(pieces[e], key=lambda p: p[2]):
            ye = res.results[core][f"y_s{j}"]             # [128, DPO, Cs]
            ye_tok = ye.transpose(2, 1, 0).reshape(caps[j], D)
            parts.append(ye_tok[:cnt])
        ye_all = np.concatenate(parts, axis=0) if len(parts) > 1 else parts[0]
        if e < E:
            y[toks] = y[toks] + ye_all
        else:
            z[toks] = ye_all
    out = (y + z).reshape(B, S, D)
    return out.astype(x.dtype)
